# revision 34
# baseline (speedup 1.0000x reference)
"""Pairwise squared L2 distance (retrieval KNN) on 8 TRN2 NeuronCores.

dist[i, j] = ||x_i||^2 + ||y_j||^2 - 2 * <x_i, y_j>

Sharding: rows of x are split across the 8 cores (data-parallel over n);
y is replicated. Each core computes a [1024, 8192] slab of the distance
matrix.

The kernel is memory-bound on the output stores (measured ~305 GB/s
per-core HBM under 8-core load), so the slab is computed and stored in
fp16 (dist magnitudes are 85..500, fp16 keeps relative error ~1e-3
against the 2e-2 gate) and upcast to fp32 on the host. That halves HBM
store traffic vs fp32 (16 MiB/core) and sets the DMA roofline at
~63 us/core.

The cross term runs as a single fp16 matmul per [128,512] tile (x
pre-scaled by -2 on the host, so PSUM accumulates -2<x,y> directly).
The PE streams rows at 216-427 ns per tile (power-managed duty
cycling), so the 128 matmuls cost ~45 us — under the DMA roofline as
long as the PE never stalls. To keep every engine streaming, PSUM is
split into 2-bank groups (1024 cols) with 4 buffers in flight, and the
norm-term epilogue is interleaved fine-grained across ScalarE and
VectorE (measured: coarser/bunched op schedules trigger the power
throttle and SBUF contention, inflating every engine ~25%):
  - 1 in 4 groups: fused VectorE scalar_tensor_tensor
    out = (psum + xsq) + ysq_bcast  (~1.28 us)
  - 3 in 4 groups: ScalarE a = psum + xsq (per-partition bias, fp16
    out, ~1.13 us, frees PSUM) then an all-fp16 2x-mode VectorE add of
    the ysq broadcast (~0.69 us).
ysq_bcast ([128, M] fp16) is a HOST-PREPARED input: building it
on-chip with gpsimd partition_broadcast gated the first store to
~24 us and its SBUF traffic inflated concurrent VectorE ops; loading
it costs 2 MiB of DMA in the otherwise-idle head. A [128, 2048] fp16
tile collects each block's two groups so stores stay 1 MiB each; the
final two blocks store in 1024-col halves to shorten the drain. ~12
warm-up matmuls on a memset scratch tile ramp the PE p-state during
the load window. Loads are issued from three engine queues in
parallel. All transposes / fp16 casts / norm vectors are host-side.
"""

import numpy as np

import concourse.bass as bass
import concourse.mybir as mybir
import concourse.tile as tile
from concourse import bacc
from concourse.bass import ts
from concourse.bass_utils import run_bass_kernel_spmd

N, M, D = 8192, 8192, 128
NCORES = 8
SLAB = N // NCORES  # 1024 rows of x per core
P = 128  # partitions / m-chunk height
MCH = SLAB // P  # 8 m-chunks per core
NT = 512  # matmul free-dim tile (one fp32 PSUM bank)
GW = 2  # n-chunks per PSUM group (2 banks = 4 KiB/partition)
GCOLS = GW * NT  # 1024
SCOLS = 2048  # store tile width (two PSUM groups)
NG = M // SCOLS  # 4 store column groups
LW = 2048  # y load-chunk width
YC = M // LW  # 4 load chunks

_f32 = mybir.dt.float32
_f16 = mybir.dt.float16
_IDENT = mybir.ActivationFunctionType.Identity
_ADD = mybir.AluOpType.add

_compiled_nc = None


def _build():
    """Build + compile the single-core Bass program (SPMD across 8 cores)."""
    nc = bacc.Bacc(
        "TRN2",
        target_bir_lowering=False,
        debug=False,
        enable_asserts=False,
        num_devices=NCORES,
    )
    xm2 = nc.dram_tensor("xm2", [D, SLAB], _f16, kind="ExternalInput").ap()
    yh = nc.dram_tensor("yh", [D, M], _f16, kind="ExternalInput").ap()
    ysqb = nc.dram_tensor("ysqb", [P, M], _f16, kind="ExternalInput").ap()
    xsq = nc.dram_tensor("xsq", [P, MCH], _f32, kind="ExternalInput").ap()
    dist16 = nc.dram_tensor("dist16", [SLAB, M], _f16, kind="ExternalOutput").ap()

    with tile.TileContext(nc) as tc:
        with (
            tc.tile_pool(name="consts", bufs=1) as cpool,
            tc.tile_pool(name="psum", bufs=4, space="PSUM") as pspool,
            tc.tile_pool(name="abuf", bufs=6) as apool,
            tc.tile_pool(name="obuf", bufs=6) as opool,
        ):
            # First-group inputs lead so the PE can start ASAP. Issue the
            # first loads from three different engine queues in parallel
            # (HWDGE issue is ~600 ns serialized per queue).
            # The first column group's y, ysq_b and x lead so the PE and the
            # epilogue engines can start ASAP; the rest of y and ysq_b
            # interleave in 2048-col chunks on the sync queue.
            yh_sb = cpool.tile([D, M], _f16)
            ysq_b = cpool.tile([P, M], _f16)
            nc.sync.dma_start(yh_sb[:, 0:NT], yh[:, 0:NT])
            xm2_sb = cpool.tile([D, SLAB], _f16)
            nc.scalar.dma_start(xm2_sb[:], xm2[:])
            xsq_sb = cpool.tile([P, MCH], _f32)
            nc.scalar.dma_start(xsq_sb[:], xsq[:])
            nc.sync.dma_start(yh_sb[:, NT:GCOLS], yh[:, NT:GCOLS])
            nc.gpsimd.dma_start(ysq_b[:, 0:GCOLS], ysqb[:, 0:GCOLS])
            # Tail of y / ysq_b as single wide loads: 14 KB-per-row packets
            # instead of 4 KB improve DMA per-packet efficiency.
            nc.sync.dma_start(yh_sb[:, GCOLS:M], yh[:, GCOLS:M])
            nc.gpsimd.dma_start(ysq_b[:, GCOLS:M], ysqb[:, GCOLS:M])

            # Warm up the PE's p-state during the load window: ~10 matmuls
            # on a memset scratch tile ramp the clock (2.5-3 us of
            # continuous execution) so the first real tiles run full-pitch.
            warm = cpool.tile([P, NT], _f16)
            nc.vector.memset(warm[:], 0.0)
            wps = pspool.tile([P, GCOLS], _f32, tag="ps")
            for w in range(12):
                nc.tensor.matmul(
                    wps[:, ts(w % 2, NT)],
                    warm[:, 0:P],
                    warm[:],
                    start=True,
                    stop=True,
                )

            def emit_group(mc, gg, ot, h):
                """One [128, GCOLS] PSUM group -> ot[:, h*GCOLS:...]."""
                x_w = xm2_sb[:, ts(mc, P)]
                c0 = gg * GCOLS
                ps = pspool.tile([P, GCOLS], _f32, tag="ps")
                for jj in range(GW):
                    nc.tensor.matmul(
                        ps[:, ts(jj, NT)],
                        x_w,
                        yh_sb[:, c0 + jj * NT : c0 + (jj + 1) * NT],
                        start=True,
                        stop=True,
                    )
                osl = ot[:, h * GCOLS : (h + 1) * GCOLS]
                # Last block goes fused to shorten the end-of-run drain chain.
                last_block = (gg // 2 == NG - 1) and (mc == MCH - 1)
                if (2 * mc + h) % 4 == 0 or last_block:
                    # Fused VectorE epilogue: out = (psum + xsq) + ysq_b.
                    nc.vector.scalar_tensor_tensor(
                        osl,
                        ps[:],
                        xsq_sb[:, mc : mc + 1],
                        ysq_b[:, c0 : c0 + GCOLS],
                        op0=_ADD,
                        op1=_ADD,
                    )
                else:
                    # ScalarE: a = psum + xsq (frees PSUM), then an all-fp16
                    # 2x-mode VectorE add of the ysq broadcast.
                    a = apool.tile([P, GCOLS], _f16, tag="a")
                    nc.scalar.activation(
                        a[:],
                        ps[:],
                        _IDENT,
                        bias=xsq_sb[:, mc : mc + 1],
                        scale=1.0,
                    )
                    nc.vector.tensor_add(osl, a[:], ysq_b[:, c0 : c0 + GCOLS])

            for g in range(NG):
                for mc in range(MCH):
                    ot = opool.tile([P, SCOLS], _f16, tag="ot")
                    last = g == NG - 1 and mc >= MCH - 2
                    for h in range(2):
                        emit_group(mc, 2 * g + h, ot, h)
                        if last:
                            # Split the final store so the end-of-run drain
                            # transfer is half as long.
                            c0 = g * SCOLS + h * GCOLS
                            nc.sync.dma_start(
                                dist16[ts(mc, P), c0 : c0 + GCOLS],
                                ot[:, h * GCOLS : (h + 1) * GCOLS],
                            )
                    if not last:
                        nc.sync.dma_start(
                            dist16[ts(mc, P), g * SCOLS : (g + 1) * SCOLS],
                            ot[:],
                        )

    nc.compile()
    return nc


def _get_nc():
    global _compiled_nc
    if _compiled_nc is None:
        _compiled_nc = _build()
    return _compiled_nc


def make_in_maps(x: np.ndarray, y: np.ndarray) -> list[dict[str, np.ndarray]]:
    x = np.asarray(x, dtype=np.float32)
    y = np.asarray(y, dtype=np.float32)
    x_sq = np.sum(x * x, axis=1, dtype=np.float32)
    y_sq = np.sum(y * y, axis=1, dtype=np.float32)

    xm2t = (-2.0 * x).T.astype(np.float16)  # [D, N]
    yt16 = np.ascontiguousarray(y.T.astype(np.float16))  # [D, M]
    ysqb_in = np.ascontiguousarray(
        np.broadcast_to(y_sq.astype(np.float16).reshape(1, M), (P, M))
    )

    in_maps = []
    for c in range(NCORES):
        sl = slice(c * SLAB, (c + 1) * SLAB)
        # [P, MCH]: column mc holds x_sq for rows mc*128..mc*128+127
        xsq_in = np.ascontiguousarray(x_sq[sl].reshape(MCH, P).T)
        in_maps.append(
            {
                "xm2": np.ascontiguousarray(xm2t[:, sl]),
                "yh": yt16,
                "ysqb": ysqb_in,
                "xsq": xsq_in,
            }
        )
    return in_maps


def kernel(x: np.ndarray, y: np.ndarray, **run_kwargs) -> np.ndarray:
    nc = _get_nc()
    in_maps = make_in_maps(x, y)
    res = run_bass_kernel_spmd(nc, in_maps, core_ids=list(range(NCORES)), **run_kwargs)
    out = np.concatenate(
        [res.results[c]["dist16"] for c in range(NCORES)], axis=0
    ).astype(np.float32)
    if run_kwargs:
        kernel.last_results = res
    return out


# revision 36
# speedup vs baseline: 1.1081x; 1.1081x over previous
"""Pairwise squared L2 distance (retrieval KNN) on 8 TRN2 NeuronCores.

dist[i, j] = ||x_i||^2 + ||y_j||^2 - 2 * <x_i, y_j>

Sharding: rows of x are split across the 8 cores (data-parallel over n);
y is replicated. Each core computes a [1024, 8192] slab of the distance
matrix.

The kernel is memory-bound on the output stores (measured ~305 GB/s
per-core HBM under 8-core load), so the slab is computed and stored in
fp16 (dist magnitudes are 85..500, fp16 keeps relative error ~1e-3
against the 2e-2 gate) and upcast to fp32 on the host. That halves HBM
store traffic vs fp32 (16 MiB/core) and sets the DMA roofline at
~63 us/core.

The cross term runs as a single fp16 matmul per [128,512] tile (x
pre-scaled by -2 on the host, so PSUM accumulates -2<x,y> directly).
The PE streams rows at 216-427 ns per tile (power-managed duty
cycling), so the 128 matmuls cost ~45 us — under the DMA roofline as
long as the PE never stalls. To keep every engine streaming, PSUM is
split into 2-bank groups (1024 cols) with 4 buffers in flight, and the
norm-term epilogue is interleaved fine-grained across ScalarE and
VectorE (measured: coarser/bunched op schedules trigger the power
throttle and SBUF contention, inflating every engine ~25%):
  - 1 in 4 groups: fused VectorE scalar_tensor_tensor
    out = (psum + xsq) + ysq_bcast  (~1.28 us)
  - 3 in 4 groups: ScalarE a = psum + xsq (per-partition bias, fp16
    out, ~1.13 us, frees PSUM) then an all-fp16 2x-mode VectorE add of
    the ysq broadcast (~0.69 us).
ysq_bcast ([128, M] fp16) is a HOST-PREPARED input: building it
on-chip with gpsimd partition_broadcast gated the first store to
~24 us and its SBUF traffic inflated concurrent VectorE ops; loading
it costs 2 MiB of DMA in the otherwise-idle head. A [128, 2048] fp16
tile collects each block's two groups so stores stay 1 MiB each; the
final two blocks store in 1024-col halves to shorten the drain. ~12
warm-up matmuls on a memset scratch tile ramp the PE p-state during
the load window. Loads are issued from three engine queues in
parallel. All transposes / fp16 casts / norm vectors are host-side.
"""

import numpy as np

import concourse.bass as bass
import concourse.mybir as mybir
import concourse.tile as tile
from concourse import bacc
from concourse.bass import ts
from concourse.bass_utils import run_bass_kernel_spmd

N, M, D = 8192, 8192, 128
NCORES = 8
SLAB = N // NCORES  # 1024 rows of x per core
P = 128  # partitions / m-chunk height
MCH = SLAB // P  # 8 m-chunks per core
NT = 512  # matmul free-dim tile (one fp32 PSUM bank)
GW = 2  # n-chunks per PSUM group (2 banks = 4 KiB/partition)
GCOLS = GW * NT  # 1024
SCOLS = 2048  # store tile width (two PSUM groups)
NG = M // SCOLS  # 4 store column groups
LW = 2048  # y load-chunk width
YC = M // LW  # 4 load chunks

_f32 = mybir.dt.float32
_f16 = mybir.dt.float16
_IDENT = mybir.ActivationFunctionType.Identity
_ADD = mybir.AluOpType.add

_compiled_nc = None


def _build():
    """Build + compile the single-core Bass program (SPMD across 8 cores)."""
    nc = bacc.Bacc(
        "TRN2",
        target_bir_lowering=False,
        debug=False,
        enable_asserts=False,
        num_devices=NCORES,
    )
    xm2 = nc.dram_tensor("xm2", [D, SLAB], _f16, kind="ExternalInput").ap()
    yh = nc.dram_tensor("yh", [D, M], _f16, kind="ExternalInput").ap()
    ysqb = nc.dram_tensor("ysqb", [P, M], _f16, kind="ExternalInput").ap()
    xsq = nc.dram_tensor("xsq", [P, MCH], _f32, kind="ExternalInput").ap()
    dist16 = nc.dram_tensor("dist16", [SLAB, M], _f16, kind="ExternalOutput").ap()

    with tile.TileContext(nc) as tc:
        with (
            tc.tile_pool(name="consts", bufs=1) as cpool,
            tc.tile_pool(name="psum", bufs=4, space="PSUM") as pspool,
            tc.tile_pool(name="abuf", bufs=6) as apool,
            tc.tile_pool(name="obuf", bufs=6) as opool,
        ):
            # First-group inputs lead so the PE can start ASAP. Issue the
            # first loads from three different engine queues in parallel
            # (HWDGE issue is ~600 ns serialized per queue).
            # The first column group's y, ysq_b and x lead so the PE and the
            # epilogue engines can start ASAP; the rest of y and ysq_b
            # interleave in 2048-col chunks on the sync queue.
            yh_sb = cpool.tile([D, M], _f16)
            ysq_b = cpool.tile([P, M], _f16)
            nc.sync.dma_start(yh_sb[:, 0:NT], yh[:, 0:NT])
            xm2_sb = cpool.tile([D, SLAB], _f16)
            nc.scalar.dma_start(xm2_sb[:], xm2[:])
            xsq_sb = cpool.tile([P, MCH], _f32)
            nc.scalar.dma_start(xsq_sb[:], xsq[:])
            nc.sync.dma_start(yh_sb[:, NT:GCOLS], yh[:, NT:GCOLS])
            nc.gpsimd.dma_start(ysq_b[:, 0:GCOLS], ysqb[:, 0:GCOLS])
            nc.sync.dma_start(
                yh_sb[:, GCOLS : 2 * GCOLS], yh[:, GCOLS : 2 * GCOLS]
            )
            nc.gpsimd.dma_start(
                ysq_b[:, GCOLS : 2 * GCOLS], ysqb[:, GCOLS : 2 * GCOLS]
            )
            for c in range(1, YC):
                nc.sync.dma_start(yh_sb[:, ts(c, LW)], yh[:, ts(c, LW)])
                nc.gpsimd.dma_start(ysq_b[:, ts(c, LW)], ysqb[:, ts(c, LW)])

            # Warm up the PE's p-state during the load window: ~10 matmuls
            # on a memset scratch tile ramp the clock (2.5-3 us of
            # continuous execution) so the first real tiles run full-pitch.
            warm = cpool.tile([P, NT], _f16)
            nc.vector.memset(warm[:], 0.0)
            wps = pspool.tile([P, GCOLS], _f32, tag="ps")
            for w in range(12):
                nc.tensor.matmul(
                    wps[:, ts(w % 2, NT)],
                    warm[:, 0:P],
                    warm[:],
                    start=True,
                    stop=True,
                )

            def emit_group(mc, gg, ot, h):
                """One [128, GCOLS] PSUM group -> ot[:, h*GCOLS:...]."""
                x_w = xm2_sb[:, ts(mc, P)]
                c0 = gg * GCOLS
                ps = pspool.tile([P, GCOLS], _f32, tag="ps")
                for jj in range(GW):
                    nc.tensor.matmul(
                        ps[:, ts(jj, NT)],
                        x_w,
                        yh_sb[:, c0 + jj * NT : c0 + (jj + 1) * NT],
                        start=True,
                        stop=True,
                    )
                osl = ot[:, h * GCOLS : (h + 1) * GCOLS]
                # Last block goes fused to shorten the end-of-run drain chain.
                last_block = (gg // 2 == NG - 1) and (mc == MCH - 1)
                if (2 * mc + h) % 4 == 0 or (2 * mc + h) == 6 or last_block:
                    # Fused VectorE epilogue: out = (psum + xsq) + ysq_b.
                    nc.vector.scalar_tensor_tensor(
                        osl,
                        ps[:],
                        xsq_sb[:, mc : mc + 1],
                        ysq_b[:, c0 : c0 + GCOLS],
                        op0=_ADD,
                        op1=_ADD,
                    )
                else:
                    # ScalarE: a = psum + xsq (frees PSUM), then an all-fp16
                    # 2x-mode VectorE add of the ysq broadcast.
                    a = apool.tile([P, GCOLS], _f16, tag="a")
                    nc.scalar.activation(
                        a[:],
                        ps[:],
                        _IDENT,
                        bias=xsq_sb[:, mc : mc + 1],
                        scale=1.0,
                    )
                    nc.vector.tensor_add(osl, a[:], ysq_b[:, c0 : c0 + GCOLS])

            for g in range(NG):
                for mc in range(MCH):
                    ot = opool.tile([P, SCOLS], _f16, tag="ot")
                    last = g == NG - 1 and mc >= MCH - 2
                    for h in range(2):
                        emit_group(mc, 2 * g + h, ot, h)
                        if last:
                            # Split the final store so the end-of-run drain
                            # transfer is half as long.
                            c0 = g * SCOLS + h * GCOLS
                            nc.sync.dma_start(
                                dist16[ts(mc, P), c0 : c0 + GCOLS],
                                ot[:, h * GCOLS : (h + 1) * GCOLS],
                            )
                    if not last:
                        nc.sync.dma_start(
                            dist16[ts(mc, P), g * SCOLS : (g + 1) * SCOLS],
                            ot[:],
                        )

    nc.compile()
    return nc


def _get_nc():
    global _compiled_nc
    if _compiled_nc is None:
        _compiled_nc = _build()
    return _compiled_nc


def make_in_maps(x: np.ndarray, y: np.ndarray) -> list[dict[str, np.ndarray]]:
    x = np.asarray(x, dtype=np.float32)
    y = np.asarray(y, dtype=np.float32)
    x_sq = np.sum(x * x, axis=1, dtype=np.float32)
    y_sq = np.sum(y * y, axis=1, dtype=np.float32)

    xm2t = (-2.0 * x).T.astype(np.float16)  # [D, N]
    yt16 = np.ascontiguousarray(y.T.astype(np.float16))  # [D, M]
    ysqb_in = np.ascontiguousarray(
        np.broadcast_to(y_sq.astype(np.float16).reshape(1, M), (P, M))
    )

    in_maps = []
    for c in range(NCORES):
        sl = slice(c * SLAB, (c + 1) * SLAB)
        # [P, MCH]: column mc holds x_sq for rows mc*128..mc*128+127
        xsq_in = np.ascontiguousarray(x_sq[sl].reshape(MCH, P).T)
        in_maps.append(
            {
                "xm2": np.ascontiguousarray(xm2t[:, sl]),
                "yh": yt16,
                "ysqb": ysqb_in,
                "xsq": xsq_in,
            }
        )
    return in_maps


def kernel(x: np.ndarray, y: np.ndarray, **run_kwargs) -> np.ndarray:
    nc = _get_nc()
    in_maps = make_in_maps(x, y)
    res = run_bass_kernel_spmd(nc, in_maps, core_ids=list(range(NCORES)), **run_kwargs)
    out = np.concatenate(
        [res.results[c]["dist16"] for c in range(NCORES)], axis=0
    ).astype(np.float32)
    if run_kwargs:
        kernel.last_results = res
    return out


# revision 38
# speedup vs baseline: 1.1384x; 1.0273x over previous
"""Pairwise squared L2 distance (retrieval KNN) on 8 TRN2 NeuronCores.

dist[i, j] = ||x_i||^2 + ||y_j||^2 - 2 * <x_i, y_j>

Sharding: rows of x are split across the 8 cores (data-parallel over n);
y is replicated. Each core computes a [1024, 8192] slab of the distance
matrix.

The kernel is memory-bound on the output stores (measured ~305 GB/s
per-core HBM under 8-core load), so the slab is computed and stored in
fp16 (dist magnitudes are 85..500, fp16 keeps relative error ~1e-3
against the 2e-2 gate) and upcast to fp32 on the host. That halves HBM
store traffic vs fp32 (16 MiB/core) and sets the DMA roofline at
~63 us/core.

The cross term runs as a single fp16 matmul per [128,512] tile (x
pre-scaled by -2 on the host, so PSUM accumulates -2<x,y> directly).
The PE streams rows at 216-427 ns per tile (power-managed duty
cycling), so the 128 matmuls cost ~45 us — under the DMA roofline as
long as the PE never stalls. To keep every engine streaming, PSUM is
split into 2-bank groups (1024 cols) with 4 buffers in flight, and the
norm-term epilogue is interleaved fine-grained across ScalarE and
VectorE (measured: coarser/bunched op schedules trigger the power
throttle and SBUF contention, inflating every engine ~25%):
  - 1 in 4 groups: fused VectorE scalar_tensor_tensor
    out = (psum + xsq) + ysq_bcast  (~1.28 us)
  - 3 in 4 groups: ScalarE a = psum + xsq (per-partition bias, fp16
    out, ~1.13 us, frees PSUM) then an all-fp16 2x-mode VectorE add of
    the ysq broadcast (~0.69 us).
ysq_bcast ([128, M] fp16) is a HOST-PREPARED input: building it
on-chip with gpsimd partition_broadcast gated the first store to
~24 us and its SBUF traffic inflated concurrent VectorE ops; loading
it costs 2 MiB of DMA in the otherwise-idle head. A [128, 2048] fp16
tile collects each block's two groups so stores stay 1 MiB each; the
final two blocks store in 1024-col halves to shorten the drain. ~12
warm-up matmuls on a memset scratch tile ramp the PE p-state during
the load window. Loads are issued from three engine queues in
parallel. All transposes / fp16 casts / norm vectors are host-side.
"""

import numpy as np

import concourse.bass as bass
import concourse.mybir as mybir
import concourse.tile as tile
from concourse import bacc
from concourse.bass import ts
from concourse.bass_utils import run_bass_kernel_spmd

N, M, D = 8192, 8192, 128
NCORES = 8
SLAB = N // NCORES  # 1024 rows of x per core
P = 128  # partitions / m-chunk height
MCH = SLAB // P  # 8 m-chunks per core
NT = 512  # matmul free-dim tile (one fp32 PSUM bank)
GW = 2  # n-chunks per PSUM group (2 banks = 4 KiB/partition)
GCOLS = GW * NT  # 1024
SCOLS = 2048  # store tile width (two PSUM groups)
NG = M // SCOLS  # 4 store column groups
LW = 2048  # y load-chunk width
YC = M // LW  # 4 load chunks

_f32 = mybir.dt.float32
_f16 = mybir.dt.float16
_IDENT = mybir.ActivationFunctionType.Identity
_ADD = mybir.AluOpType.add

_compiled_nc = None


def _build():
    """Build + compile the single-core Bass program (SPMD across 8 cores)."""
    nc = bacc.Bacc(
        "TRN2",
        target_bir_lowering=False,
        debug=False,
        enable_asserts=False,
        num_devices=NCORES,
    )
    xm2 = nc.dram_tensor("xm2", [D, SLAB], _f16, kind="ExternalInput").ap()
    yh = nc.dram_tensor("yh", [D, M], _f16, kind="ExternalInput").ap()
    ysqb = nc.dram_tensor("ysqb", [P, M], _f16, kind="ExternalInput").ap()
    xsq = nc.dram_tensor("xsq", [P, MCH], _f32, kind="ExternalInput").ap()
    dist16 = nc.dram_tensor("dist16", [SLAB, M], _f16, kind="ExternalOutput").ap()

    with tile.TileContext(nc) as tc:
        with (
            tc.tile_pool(name="consts", bufs=1) as cpool,
            tc.tile_pool(name="psum", bufs=4, space="PSUM") as pspool,
            tc.tile_pool(name="abuf", bufs=7) as apool,
            tc.tile_pool(name="obuf", bufs=7) as opool,
        ):
            # First-group inputs lead so the PE can start ASAP. Issue the
            # first loads from three different engine queues in parallel
            # (HWDGE issue is ~600 ns serialized per queue).
            # The first column group's y, ysq_b and x lead so the PE and the
            # epilogue engines can start ASAP; the rest of y and ysq_b
            # interleave in 2048-col chunks on the sync queue.
            yh_sb = cpool.tile([D, M], _f16)
            ysq_b = cpool.tile([P, M], _f16)
            nc.sync.dma_start(yh_sb[:, 0:NT], yh[:, 0:NT])
            xm2_sb = cpool.tile([D, SLAB], _f16)
            nc.scalar.dma_start(xm2_sb[:], xm2[:])
            xsq_sb = cpool.tile([P, MCH], _f32)
            nc.scalar.dma_start(xsq_sb[:], xsq[:])
            nc.sync.dma_start(yh_sb[:, NT:GCOLS], yh[:, NT:GCOLS])
            nc.gpsimd.dma_start(ysq_b[:, 0:GCOLS], ysqb[:, 0:GCOLS])
            nc.sync.dma_start(
                yh_sb[:, GCOLS : 2 * GCOLS], yh[:, GCOLS : 2 * GCOLS]
            )
            nc.gpsimd.dma_start(
                ysq_b[:, GCOLS : 2 * GCOLS], ysqb[:, GCOLS : 2 * GCOLS]
            )
            for c in range(1, YC):
                nc.sync.dma_start(yh_sb[:, ts(c, LW)], yh[:, ts(c, LW)])
                nc.gpsimd.dma_start(ysq_b[:, ts(c, LW)], ysqb[:, ts(c, LW)])

            # Warm up the PE's p-state during the load window: ~10 matmuls
            # on a memset scratch tile ramp the clock (2.5-3 us of
            # continuous execution) so the first real tiles run full-pitch.
            warm = cpool.tile([P, NT], _f16)
            nc.vector.memset(warm[:], 0.0)
            wps = pspool.tile([P, GCOLS], _f32, tag="ps")
            for w in range(12):
                nc.tensor.matmul(
                    wps[:, ts(w % 2, NT)],
                    warm[:, 0:P],
                    warm[:],
                    start=True,
                    stop=True,
                )

            def emit_group(mc, gg, ot, h):
                """One [128, GCOLS] PSUM group -> ot[:, h*GCOLS:...]."""
                x_w = xm2_sb[:, ts(mc, P)]
                c0 = gg * GCOLS
                ps = pspool.tile([P, GCOLS], _f32, tag="ps")
                for jj in range(GW):
                    nc.tensor.matmul(
                        ps[:, ts(jj, NT)],
                        x_w,
                        yh_sb[:, c0 + jj * NT : c0 + (jj + 1) * NT],
                        start=True,
                        stop=True,
                    )
                osl = ot[:, h * GCOLS : (h + 1) * GCOLS]
                # Last block goes fused to shorten the end-of-run drain chain.
                last_block = (gg // 2 == NG - 1) and (mc == MCH - 1)
                if (2 * mc + h) % 4 == 0 or last_block:
                    # Fused VectorE epilogue: out = (psum + xsq) + ysq_b.
                    nc.vector.scalar_tensor_tensor(
                        osl,
                        ps[:],
                        xsq_sb[:, mc : mc + 1],
                        ysq_b[:, c0 : c0 + GCOLS],
                        op0=_ADD,
                        op1=_ADD,
                    )
                else:
                    # ScalarE: a = psum + xsq (frees PSUM), then an all-fp16
                    # 2x-mode VectorE add of the ysq broadcast.
                    a = apool.tile([P, GCOLS], _f16, tag="a")
                    nc.scalar.activation(
                        a[:],
                        ps[:],
                        _IDENT,
                        bias=xsq_sb[:, mc : mc + 1],
                        scale=1.0,
                    )
                    nc.vector.tensor_add(osl, a[:], ysq_b[:, c0 : c0 + GCOLS])

            for g in range(NG):
                for mc in range(MCH):
                    ot = opool.tile([P, SCOLS], _f16, tag="ot")
                    last = g == NG - 1 and mc >= MCH - 2
                    for h in range(2):
                        emit_group(mc, 2 * g + h, ot, h)
                        if last:
                            # Split the final store so the end-of-run drain
                            # transfer is half as long.
                            c0 = g * SCOLS + h * GCOLS
                            nc.sync.dma_start(
                                dist16[ts(mc, P), c0 : c0 + GCOLS],
                                ot[:, h * GCOLS : (h + 1) * GCOLS],
                            )
                    if not last:
                        nc.sync.dma_start(
                            dist16[ts(mc, P), g * SCOLS : (g + 1) * SCOLS],
                            ot[:],
                        )

    nc.compile()
    return nc


def _get_nc():
    global _compiled_nc
    if _compiled_nc is None:
        _compiled_nc = _build()
    return _compiled_nc


def make_in_maps(x: np.ndarray, y: np.ndarray) -> list[dict[str, np.ndarray]]:
    x = np.asarray(x, dtype=np.float32)
    y = np.asarray(y, dtype=np.float32)
    x_sq = np.sum(x * x, axis=1, dtype=np.float32)
    y_sq = np.sum(y * y, axis=1, dtype=np.float32)

    xm2t = (-2.0 * x).T.astype(np.float16)  # [D, N]
    yt16 = np.ascontiguousarray(y.T.astype(np.float16))  # [D, M]
    ysqb_in = np.ascontiguousarray(
        np.broadcast_to(y_sq.astype(np.float16).reshape(1, M), (P, M))
    )

    in_maps = []
    for c in range(NCORES):
        sl = slice(c * SLAB, (c + 1) * SLAB)
        # [P, MCH]: column mc holds x_sq for rows mc*128..mc*128+127
        xsq_in = np.ascontiguousarray(x_sq[sl].reshape(MCH, P).T)
        in_maps.append(
            {
                "xm2": np.ascontiguousarray(xm2t[:, sl]),
                "yh": yt16,
                "ysqb": ysqb_in,
                "xsq": xsq_in,
            }
        )
    return in_maps


def kernel(x: np.ndarray, y: np.ndarray, **run_kwargs) -> np.ndarray:
    nc = _get_nc()
    in_maps = make_in_maps(x, y)
    res = run_bass_kernel_spmd(nc, in_maps, core_ids=list(range(NCORES)), **run_kwargs)
    out = np.concatenate(
        [res.results[c]["dist16"] for c in range(NCORES)], axis=0
    ).astype(np.float32)
    if run_kwargs:
        kernel.last_results = res
    return out
